# revision 1
# baseline (speedup 1.0000x reference)
"""Grouped 2-layer MLP (ConvNN) Trainium2 kernel.

Math (per group g of SIZE=2048):
    h[b,g,:]   = LeakyReLU_0.2(W0[g] @ x[b] + b0[g])     (64 -> 64)
    out[b,g,:] = W1[g] @ h[b,g,:] + b1[g]                (64 -> 64)

Strategy:
  - Shard the group axis over 8 cores (256 groups/core), fully independent.
  - Host pre-transposes weights to [g, k, j] and x to xT so that both
    layers run on the PE with the contraction dim on partitions and the
    batch dim (1024) streaming as the moving operand. No on-chip
    transposes anywhere.
  - Groups are processed in pairs stacked on the 128 SBUF partitions;
    the two 64x64 matmuls of a pair land in opposite quadrants of the
    128x128 PE array and run concurrently (tile_position auto-derived
    from base partitions).
  - fp32r matmuls (1 PE cycle/row at N=512 vs 4 for fp32, ~11-bit
    mantissa). Walrus only allows ONE semaphore wait on an fp32r
    matmul (fused LDWEIGHTS struct), so deps are arranged to need at
    most one: weights+xT are staged via DMA then DVE-copied (with
    fp32r rounding) into resident tiles, making every mm0 dependency a
    DVE-sem tick; a single alternating PSUM tag makes every mm1
    dependency an ACT-sem tick.
  - h never touches HBM. Layer-0 bias+LeakyReLU is a single ScalarE
    activation (PSUM->SBUF, fp32r out); layer-1 bias add is a single
    VectorE tensor_scalar (PSUM->SBUF), splitting the two PSUM
    evacuation passes across ACT and DVE.
  - Output is written as [pair, 128, B] (contiguous 512KB DMA per pair)
    and un-transposed on the host.
"""

from contextlib import ExitStack

import numpy as np

import concourse.bass as bass
import concourse.mybir as mybir
import concourse.tile as tile
from concourse.bass_utils import run_bass_kernel_spmd

B = 1024
IN_DIM = 64
SIZE = 2048
D1 = 64
D2 = 64
NEG_SLOPE = 0.2
N_CORES = 8
GPC = SIZE // N_CORES  # 256 groups per core
NPAIR = GPC // 2  # 128 group-pairs per core
WB = 8  # group-pairs per weight DMA chunk
NCHUNK = NPAIR // WB

_NC_CACHE = None


def _build():
    global _NC_CACHE
    if _NC_CACHE is not None:
        return _NC_CACHE

    f32 = mybir.dt.float32
    f16 = mybir.dt.float16

    nc = bass.Bass()
    xT2 = nc.declare_dram_parameter("xT2", [128, B], f16, isOutput=False)
    w0t = nc.declare_dram_parameter("w0t", [128, NPAIR, D1], f16, isOutput=False)
    w1t = nc.declare_dram_parameter("w1t", [128, NPAIR, D2], f16, isOutput=False)
    b0p = nc.declare_dram_parameter("b0p", [128, NPAIR], f32, isOutput=False)
    b1p = nc.declare_dram_parameter("b1p", [128, NPAIR], f32, isOutput=False)
    out = nc.declare_dram_parameter("out", [NPAIR, 128, B], f32, isOutput=True)

    with ExitStack() as ctx:
        tc = ctx.enter_context(tile.TileContext(nc))
        singles = ctx.enter_context(tc.tile_pool(name="singles", bufs=1))
        stage = ctx.enter_context(tc.tile_pool(name="stage", bufs=3))
        hpool = ctx.enter_context(tc.tile_pool(name="hpool", bufs=3))
        opool = ctx.enter_context(tc.tile_pool(name="opool", bufs=3))
        pspool = ctx.enter_context(tc.tile_pool(name="psum", bufs=8, space="PSUM"))

        # biases, loaded once
        b0sb = singles.tile([128, NPAIR], f32)
        nc.sync.dma_start(out=b0sb, in_=b0p[:])
        b1sb = singles.tile([128, NPAIR], f32)
        nc.sync.dma_start(out=b1sb, in_=b1p[:])

        # xT resident, fp16 straight from HBM
        xt = singles.tile([128, B], f16)
        nc.sync.dma_start(out=xt, in_=xT2[:])

        # Weights as 128x128 block-diagonal fp32r tiles: one standard
        # K=128/M=128 matmul computes both groups of a pair (off-diagonal
        # zeros kill the cross terms; matmul time is N-driven so the
        # zeros cost nothing).  fp32r cannot col-tile (dst partition must
        # start at 0), which rules out 2x64-quadrant packing.
        # All weights land in SBUF via one contiguous 4MB DMA per layer;
        # ACT (w0) and DVE (w1) refresh only the diagonal blocks of
        # 2*WB ping-pong slots (with the fp32->fp32r rounding cast), so
        # block building for chunk k+1 overlaps PE compute on chunk k.
        w0s_all = singles.tile([128, NPAIR, D1], f16)
        nc.sync.dma_start(out=w0s_all, in_=w0t[:])
        w1s_all = singles.tile([128, NPAIR, D2], f16)
        nc.sync.dma_start(out=w1s_all, in_=w1t[:])

        w0d = singles.tile([128, 2 * WB, 128], f16)
        w1d = singles.tile([128, 2 * WB, 128], f16)
        nc.gpsimd.memset(w0d, 0.0)
        nc.gpsimd.memset(w1d, 0.0)

        for cki in range(NCHUNK):
            ck = bass.ts(cki, WB)
            half = slice((cki % 2) * WB, (cki % 2) * WB + WB)
            nc.scalar.copy(w0d[0:64, half, 0:64], w0s_all[0:64, ck, :])
            nc.scalar.copy(w0d[64:128, half, 64:128], w0s_all[64:128, ck, :])
            nc.scalar.copy(w1d[0:64, half, 0:64], w1s_all[0:64, ck, :])
            nc.scalar.copy(w1d[64:128, half, 64:128], w1s_all[64:128, ck, :])
            for q in range(WB):
                t = cki * WB + q
                qs = (cki % 2) * WB + q
                # half-pair (N=512) pipeline: 1-bank PSUM tiles so each
                # ACT/mm1/DVE stage starts as soon as half the batch is
                # ready -- keeps the dependency chain inside the 2-pair
                # PSUM lookahead and the PE dense (HAM stays warm).
                hps = [
                    pspool.tile([128, 512], f32, tag="ps", name=f"hps{t}_{i}")
                    for i in range(2)
                ]
                hsb = [
                    hpool.tile([128, 512], f16, tag=f"h{i}", name=f"hsb{t}_{i}")
                    for i in range(2)
                ]
                ops_ = [
                    pspool.tile([128, 512], f32, tag="ps", name=f"ops{t}_{i}")
                    for i in range(2)
                ]
                osb = opool.tile([128, B], f32, tag="o")
                for nb in range(2):
                    s = bass.ts(nb, 512)
                    nc.tensor.matmul(
                        hps[nb], w0d[:, qs, :], xt[:, s], start=True, stop=True
                    )
                    nc.scalar.activation(
                        out=hsb[nb],
                        in_=hps[nb],
                        func=mybir.ActivationFunctionType.Prelu,
                        bias=b0sb[:, t : t + 1],
                        scale=1.0,
                        alpha=NEG_SLOPE,
                    )
                    nc.tensor.matmul(
                        ops_[nb], w1d[:, qs, :], hsb[nb], start=True, stop=True
                    )
                    nc.vector.tensor_scalar_add(
                        osb[:, s], ops_[nb], b1sb[:, t : t + 1]
                    )
                nc.sync.dma_start(out=out[t], in_=osb)

    _split_multi_waits(nc)
    _NC_CACHE = nc
    return nc


def _split_multi_waits(nc):
    """Walrus in this toolchain allows at most ONE semaphore wait per
    instruction (and zero on the fused fp32 LDWEIGHTS struct).  Hoist all
    but the last wait of any multi-wait instruction onto same-engine NoOp
    carriers inserted directly before it — semantically identical (engine
    queues are in-order) and each carrier holds a single wait."""
    import bass_rust

    n = 0
    for f in nc.m.functions:
        for bb in f.blocks:
            out_insts = []
            changed = False
            for inst in bb.instructions:
                si = inst.sync_info
                waits = list(si.on_wait) if si is not None and si.on_wait else []
                if len(waits) > 1:
                    changed = True
                    for w in waits[:-1]:
                        nop = bass_rust.InstNoOp(
                            name=f"{inst.name}-sw{n}", engine=inst.engine
                        )
                        n += 1
                        nop.sync_info = mybir.SyncInfo(on_wait=[w], on_update=[])
                        out_insts.append(nop)
                    inst.sync_info = mybir.SyncInfo(
                        on_wait=[waits[-1]],
                        on_update=list(si.on_update) if si.on_update else [],
                    )
                out_insts.append(inst)
            if changed:
                bb.instructions = out_insts


def _prepare_in_maps(x, W0, b0, W1, b1):
    x = np.ascontiguousarray(np.asarray(x, dtype=np.float32))
    xT = x.T  # (64, 1024)
    xT2 = np.ascontiguousarray(
        np.concatenate([xT, xT], axis=0).astype(np.float16)
    )  # (128, 1024)
    in_maps = []
    for c in range(N_CORES):
        sl = slice(c * GPC, (c + 1) * GPC)
        W0c = np.asarray(W0[sl], dtype=np.float32)  # (256, 64, 64) [g, j, k]
        W1c = np.asarray(W1[sl], dtype=np.float32)
        # [g, k, j] then pair-stack on partitions: (NPAIR, 128, 64)
        w0t = np.ascontiguousarray(
            W0c.transpose(0, 2, 1).reshape(NPAIR, 128, D1).transpose(1, 0, 2)
        ).astype(np.float16)
        w1t = np.ascontiguousarray(
            W1c.transpose(0, 2, 1).reshape(NPAIR, 128, D2).transpose(1, 0, 2)
        ).astype(np.float16)
        b0p = np.ascontiguousarray(
            np.asarray(b0[sl], dtype=np.float32).reshape(NPAIR, 128).T
        )  # (128, NPAIR)
        b1p = np.ascontiguousarray(
            np.asarray(b1[sl], dtype=np.float32).reshape(NPAIR, 128).T
        )
        in_maps.append({"xT2": xT2, "w0t": w0t, "w1t": w1t, "b0p": b0p, "b1p": b1p})
    return in_maps


def _postprocess(results):
    outs = []
    for c in range(N_CORES):
        o = results[c]["out"]  # (NPAIR, 128, B) = [t, q*64+j, b]
        o = o.reshape(NPAIR, 2, 64, B).transpose(3, 0, 1, 2).reshape(B, GPC, D2)
        outs.append(o)
    return np.ascontiguousarray(np.concatenate(outs, axis=1))


def _run(inputs, trace=False):
    nc = _build()
    in_maps = _prepare_in_maps(**inputs)
    res = run_bass_kernel_spmd(
        nc, in_maps, core_ids=list(range(N_CORES)), trace=trace
    )
    return _postprocess(res.results), res


def kernel(x, W0, b0, W1, b1):
    out, _ = _run({"x": x, "W0": W0, "b0": b0, "W1": W1, "b1": b1})
    return out



# revision 7
# speedup vs baseline: 1.8622x; 1.8622x over previous
"""Grouped 2-layer MLP (ConvNN) Trainium2 kernel.

Math (per group g of SIZE=2048):
    h[b,g,:]   = LeakyReLU_0.2(W0[g] @ x[b] + b0[g])     (64 -> 64)
    out[b,g,:] = W1[g] @ h[b,g,:] + b1[g]                (64 -> 64)

Strategy (v2 — evacuation-balanced pipeline):
  - Shard the group axis over 8 cores (256 groups/core = 128 pairs),
    fully independent, no collectives.
  - Per pair t the dataflow is
        mm0 (PE) -> hps (PSUM fp32) -> Prelu (ACT) -> hsb (SBUF f16)
        mm1 (PE) -> ops (PSUM fp32) -> +b1  (DVE) -> osb (SBUF f16) -> DMA
    On TRN2 matmul output must be fp32, so each evacuation pass runs at
    1x (ACT (FD+352)/1.2ns, DVE (FD+120)/0.96ns).  With FD=1024 per-pair
    ops the engine budgets per core are PE 109us / ACT 147us / DVE 153us
    / DMA 108us, all overlapped -> DVE-paced wall ~160us.
  - Layer-0 bias rides the ACT Prelu's per-partition bias operand
    (per-pair ops make that legal), exactly as the proven baseline did.
  - Layer-1 stationary is a host-built 128x128 block-diagonal (2 groups
    per pair); off-diagonal zeros kill cross terms and cost nothing
    since matmul time is N-driven.
  - Weights are host-packed exactly in stationary layout and DMA'd in 8
    chunks of 16 pairs, prefetched 2 chunks ahead and interleaved with
    output DMAs on the FIFO sync ring, so compute starts after ~2us and
    input loads never block the output stream.
  - PE instruction stream is software-pipelined (mm0(t) emitted before
    mm1(t-1)) so the PE never sits behind the ACT dependency; gaps stay
    well under the HAM idle window so the PE runs warm at 2.4GHz.
  - PSUM: 4 rotating [128,1024] fp32 tiles (2 banks each) = all 8 banks:
    hps(t), ops(t-1) per iteration, double-buffered.
  - fp16 output (32MB/core instead of 64) is widened to fp32 on host.
"""

from contextlib import ExitStack

import numpy as np

import concourse.bass as bass
import concourse.mybir as mybir
import concourse.tile as tile
from concourse.bass_utils import run_bass_kernel_spmd

B = 1024
IN_DIM = 64
SIZE = 2048
D1 = 64
D2 = 64
NEG_SLOPE = 0.2
N_CORES = 8
GPC = SIZE // N_CORES  # 256 groups per core
NPAIR = GPC // 2  # 128 group-pairs per core
CH = 16  # pairs per weight DMA chunk
NCH = NPAIR // CH

_NC_CACHE = None
_SIM_RELU = False  # CoreSim has no Prelu; debug builds swap in Relu
_SKIP_SPLIT = False  # sim-only: skip the walrus single-wait workaround


def _build():
    global _NC_CACHE
    if _NC_CACHE is not None:
        return _NC_CACHE

    f32 = mybir.dt.float32
    f16 = mybir.dt.float16

    nc = bass.Bass()
    xt1 = nc.declare_dram_parameter("xt1", [IN_DIM, B], f16, isOutput=False)
    w0t = nc.declare_dram_parameter("w0t", [IN_DIM, NPAIR, 128], f16, isOutput=False)
    b0p = nc.declare_dram_parameter("b0p", [128, NPAIR], f32, isOutput=False)
    w1t = nc.declare_dram_parameter("w1t", [128, NPAIR, 128], f16, isOutput=False)
    b1p = nc.declare_dram_parameter("b1p", [128, NPAIR], f32, isOutput=False)
    out = nc.declare_dram_parameter("out", [NPAIR, 128, B], f16, isOutput=True)

    with ExitStack() as ctx:
        tc = ctx.enter_context(tile.TileContext(nc))
        singles = ctx.enter_context(tc.tile_pool(name="singles", bufs=1))
        hpool = ctx.enter_context(tc.tile_pool(name="hpool", bufs=4))
        opool = ctx.enter_context(tc.tile_pool(name="opool", bufs=4))
        pspool = ctx.enter_context(tc.tile_pool(name="psum", bufs=4, space="PSUM"))

        b1sb = singles.tile([128, NPAIR], f32)
        nc.sync.dma_start(out=b1sb, in_=b1p[:])
        b0sb = singles.tile([128, NPAIR], f32)
        nc.sync.dma_start(out=b0sb, in_=b0p[:])
        xt = singles.tile([IN_DIM, B], f16)
        nc.sync.dma_start(out=xt, in_=xt1[:])

        w0sb = singles.tile([IN_DIM, NPAIR, 128], f16)
        w1sb = singles.tile([128, NPAIR, 128], f16)

        def load_chunk(k):
            ck = bass.ts(k, CH)
            nc.sync.dma_start(out=w0sb[:, ck, :], in_=w0t[:, ck, :])
            nc.sync.dma_start(out=w1sb[:, ck, :], in_=w1t[:, ck, :])

        load_chunk(0)
        load_chunk(1)

        hsb = None
        for t in range(NPAIR + 1):
            # keep weight chunks ~2 ahead of the consuming pairs
            if t % CH == 0 and t // CH + 2 < NCH:
                load_chunk(t // CH + 2)
            if t < NPAIR:
                hps = pspool.tile([128, B], f32, tag="ps", name=f"hps{t}")
                for nb in range(2):
                    s = bass.ts(nb, 512)
                    nc.tensor.matmul(
                        hps[:, s], w0sb[:, t, :], xt[:, s], start=True, stop=True
                    )
            if t >= 1:
                tp = t - 1
                ops = pspool.tile([128, B], f32, tag="ps", name=f"ops{tp}")
                for nb in range(2):
                    s = bass.ts(nb, 512)
                    nc.tensor.matmul(
                        ops[:, s], w1sb[:, tp, :], hsb[:, s], start=True, stop=True
                    )
            if t < NPAIR:
                hsb_new = hpool.tile([128, B], f16, tag="h", name=f"hsb{t}")
                nc.scalar.activation(
                    out=hsb_new,
                    in_=hps,
                    func=mybir.ActivationFunctionType.Relu
                    if _SIM_RELU
                    else mybir.ActivationFunctionType.Prelu,
                    bias=b0sb[:, t : t + 1],
                    scale=1.0,
                    alpha=NEG_SLOPE,
                )
            if t >= 1:
                osb = opool.tile([128, B], f16, tag="o", name=f"osb{tp}")
                nc.vector.tensor_scalar_add(osb, ops, b1sb[:, tp : tp + 1])
                nc.sync.dma_start(out=out[tp], in_=osb)
            if t < NPAIR:
                hsb = hsb_new

    if not _SKIP_SPLIT:
        _split_multi_waits(nc)
    _NC_CACHE = nc
    return nc


def _split_multi_waits(nc):
    """Walrus in this toolchain allows at most ONE semaphore wait per
    instruction (and zero on the fused fp32 LDWEIGHTS struct).  Hoist all
    but the last wait of any multi-wait instruction onto same-engine NoOp
    carriers inserted directly before it — semantically identical (engine
    queues are in-order) and each carrier holds a single wait."""
    import bass_rust

    n = 0
    for f in nc.m.functions:
        for bb in f.blocks:
            out_insts = []
            changed = False
            for inst in bb.instructions:
                si = inst.sync_info
                waits = list(si.on_wait) if si is not None and si.on_wait else []
                if len(waits) > 1:
                    changed = True
                    for w in waits[:-1]:
                        nop = bass_rust.InstNoOp(
                            name=f"{inst.name}-sw{n}", engine=inst.engine
                        )
                        n += 1
                        nop.sync_info = mybir.SyncInfo(on_wait=[w], on_update=[])
                        out_insts.append(nop)
                    inst.sync_info = mybir.SyncInfo(
                        on_wait=[waits[-1]],
                        on_update=list(si.on_update) if si.on_update else [],
                    )
                out_insts.append(inst)
            if changed:
                bb.instructions = out_insts
    return nc


def _prepare_in_maps(x, W0, b0, W1, b1):
    x = np.asarray(x, dtype=np.float32)
    xt1 = np.ascontiguousarray(x.T.astype(np.float16))

    in_maps = []
    for c in range(N_CORES):
        sl = slice(c * GPC, (c + 1) * GPC)
        W0c = np.asarray(W0[sl], dtype=np.float32)  # (256, 64, 64) [g, j, k]
        W1c = np.asarray(W1[sl], dtype=np.float32)
        b0c = np.asarray(b0[sl], dtype=np.float32)  # (256, 64)
        b1c = np.asarray(b1[sl], dtype=np.float32)

        # w0t[k, t, q*64+j] = W0[2t+q, j, k]
        w0 = np.ascontiguousarray(
            W0c.transpose(2, 0, 1).reshape(IN_DIM, NPAIR, 128).astype(np.float16)
        )

        # w1t[q*64+k, t, q'*64+j] = W1[2t+q, j, k] iff q == q'
        w1k = W1c.transpose(2, 0, 1).reshape(D1, NPAIR, 2, D2)  # [k, t, q, j]
        w1 = np.zeros((2, D1, NPAIR, 2, D2), dtype=np.float16)
        for q in range(2):
            w1[q, :, :, q, :] = w1k[:, :, q, :].astype(np.float16)
        w1 = np.ascontiguousarray(w1.reshape(128, NPAIR, 128))

        b0pp = np.ascontiguousarray(b0c.reshape(NPAIR, 128).T)  # (128, NPAIR)
        b1pp = np.ascontiguousarray(b1c.reshape(NPAIR, 128).T)
        in_maps.append(
            {"xt1": xt1, "w0t": w0, "w1t": w1, "b0p": b0pp, "b1p": b1pp}
        )
    return in_maps


def _postprocess(results):
    outs = []
    for c in range(N_CORES):
        o = results[c]["out"]  # (NPAIR, 128, B) f16 = [t, q*64+j, b]
        o = (
            o.astype(np.float32)
            .reshape(NPAIR, 2, D2, B)
            .transpose(3, 0, 1, 2)
            .reshape(B, GPC, D2)
        )
        outs.append(o)
    return np.ascontiguousarray(np.concatenate(outs, axis=1))


def _run(inputs, trace=False):
    nc = _build()
    in_maps = _prepare_in_maps(**inputs)
    res = run_bass_kernel_spmd(
        nc, in_maps, core_ids=list(range(N_CORES)), trace=trace
    )
    return _postprocess(res.results), res


def kernel(x, W0, b0, W1, b1):
    out, _ = _run({"x": x, "W0": W0, "b0": b0, "W1": W1, "b1": b1})
    return out


# revision 8
# speedup vs baseline: 1.8625x; 1.0002x over previous
"""Grouped 2-layer MLP (ConvNN) Trainium2 kernel.

Math (per group g of SIZE=2048):
    h[b,g,:]   = LeakyReLU_0.2(W0[g] @ x[b] + b0[g])     (64 -> 64)
    out[b,g,:] = W1[g] @ h[b,g,:] + b1[g]                (64 -> 64)

Strategy (v2 — evacuation-balanced pipeline):
  - Shard the group axis over 8 cores (256 groups/core = 128 pairs),
    fully independent, no collectives.
  - Per pair t the dataflow is
        mm0 (PE) -> hps (PSUM fp32) -> Prelu (ACT) -> hsb (SBUF f16)
        mm1 (PE) -> ops (PSUM fp32) -> +b1  (DVE) -> osb (SBUF f16) -> DMA
    On TRN2 matmul output must be fp32, so each evacuation pass runs at
    1x (ACT (FD+352)/1.2ns, DVE (FD+120)/0.96ns).  With FD=1024 per-pair
    ops the engine budgets per core are PE 109us / ACT 147us / DVE 153us
    / DMA 108us, all overlapped -> DVE-paced wall ~160us.
  - Layer-0 bias rides the ACT Prelu's per-partition bias operand
    (per-pair ops make that legal), exactly as the proven baseline did.
  - Layer-1 stationary is a host-built 128x128 block-diagonal (2 groups
    per pair); off-diagonal zeros kill cross terms and cost nothing
    since matmul time is N-driven.
  - Weights are host-packed exactly in stationary layout and DMA'd in 8
    chunks of 16 pairs, prefetched 2 chunks ahead and interleaved with
    output DMAs on the FIFO sync ring, so compute starts after ~2us and
    input loads never block the output stream.
  - PE instruction stream is software-pipelined (mm0(t) emitted before
    mm1(t-1)) so the PE never sits behind the ACT dependency; gaps stay
    well under the HAM idle window so the PE runs warm at 2.4GHz.
  - PSUM: 4 rotating [128,1024] fp32 tiles (2 banks each) = all 8 banks:
    hps(t), ops(t-1) per iteration, double-buffered.
  - fp16 output (32MB/core instead of 64) is widened to fp32 on host.
"""

from contextlib import ExitStack

import numpy as np
from ml_dtypes import bfloat16

import concourse.bass as bass
import concourse.mybir as mybir
import concourse.tile as tile
from concourse.bass_utils import run_bass_kernel_spmd

B = 1024
IN_DIM = 64
SIZE = 2048
D1 = 64
D2 = 64
NEG_SLOPE = 0.2
N_CORES = 8
GPC = SIZE // N_CORES  # 256 groups per core
NPAIR = GPC // 2  # 128 group-pairs per core
CH = 16  # pairs per weight DMA chunk
NCH = NPAIR // CH

_NC_CACHE = None
_SIM_RELU = False  # CoreSim has no Prelu; debug builds swap in Relu
_SKIP_SPLIT = False  # sim-only: skip the walrus single-wait workaround


def _build():
    global _NC_CACHE
    if _NC_CACHE is not None:
        return _NC_CACHE

    f32 = mybir.dt.float32
    f16 = mybir.dt.float16
    bf16 = mybir.dt.bfloat16

    nc = bass.Bass()
    xt1 = nc.declare_dram_parameter("xt1", [IN_DIM, B], bf16, isOutput=False)
    w0t = nc.declare_dram_parameter("w0t", [IN_DIM, NPAIR, 128], bf16, isOutput=False)
    b0p = nc.declare_dram_parameter("b0p", [128, NPAIR], f32, isOutput=False)
    w1t = nc.declare_dram_parameter("w1t", [128, NPAIR, 128], bf16, isOutput=False)
    b1p = nc.declare_dram_parameter("b1p", [128, NPAIR], f32, isOutput=False)
    out = nc.declare_dram_parameter("out", [NPAIR, 128, B], f16, isOutput=True)

    with ExitStack() as ctx:
        tc = ctx.enter_context(tile.TileContext(nc))
        singles = ctx.enter_context(tc.tile_pool(name="singles", bufs=1))
        hpool = ctx.enter_context(tc.tile_pool(name="hpool", bufs=4))
        opool = ctx.enter_context(tc.tile_pool(name="opool", bufs=4))
        pspool = ctx.enter_context(tc.tile_pool(name="psum", bufs=4, space="PSUM"))

        b1sb = singles.tile([128, NPAIR], f32)
        nc.sync.dma_start(out=b1sb, in_=b1p[:])
        b0sb = singles.tile([128, NPAIR], f32)
        nc.sync.dma_start(out=b0sb, in_=b0p[:])
        xt = singles.tile([IN_DIM, B], bf16)
        nc.sync.dma_start(out=xt, in_=xt1[:])

        w0sb = singles.tile([IN_DIM, NPAIR, 128], bf16)
        w1sb = singles.tile([128, NPAIR, 128], bf16)

        def load_chunk(k):
            ck = bass.ts(k, CH)
            nc.sync.dma_start(out=w0sb[:, ck, :], in_=w0t[:, ck, :])
            nc.sync.dma_start(out=w1sb[:, ck, :], in_=w1t[:, ck, :])

        load_chunk(0)
        load_chunk(1)

        hsb = None
        for t in range(NPAIR + 1):
            # keep weight chunks ~2 ahead of the consuming pairs
            if t % CH == 0 and t // CH + 2 < NCH:
                load_chunk(t // CH + 2)
            if t < NPAIR:
                hps = pspool.tile([128, B], f32, tag="ps", name=f"hps{t}")
                for nb in range(2):
                    s = bass.ts(nb, 512)
                    nc.tensor.matmul(
                        hps[:, s], w0sb[:, t, :], xt[:, s], start=True, stop=True
                    )
            if t >= 1:
                tp = t - 1
                ops = pspool.tile([128, B], f32, tag="ps", name=f"ops{tp}")
                for nb in range(2):
                    s = bass.ts(nb, 512)
                    nc.tensor.matmul(
                        ops[:, s], w1sb[:, tp, :], hsb[:, s], start=True, stop=True
                    )
            if t < NPAIR:
                hsb_new = hpool.tile([128, B], bf16, tag="h", name=f"hsb{t}")
                nc.scalar.activation(
                    out=hsb_new,
                    in_=hps,
                    func=mybir.ActivationFunctionType.Relu
                    if _SIM_RELU
                    else mybir.ActivationFunctionType.Prelu,
                    bias=b0sb[:, t : t + 1],
                    scale=1.0,
                    alpha=NEG_SLOPE,
                )
            if t >= 1:
                osb = opool.tile([128, B], f16, tag="o", name=f"osb{tp}")
                if tp % 14 == 6:
                    # ~9/128 pairs evacuate layer-1 on ACT (identity+bias) to
                    # balance ACT (1114ns/op) vs DVE (1284ns/op) occupancy
                    nc.scalar.add(osb, ops, b1sb[:, tp : tp + 1])
                else:
                    nc.vector.tensor_scalar_add(osb, ops, b1sb[:, tp : tp + 1])
                nc.sync.dma_start(out=out[tp], in_=osb)
            if t < NPAIR:
                hsb = hsb_new

    if not _SKIP_SPLIT:
        _split_multi_waits(nc)
    _NC_CACHE = nc
    return nc


def _split_multi_waits(nc):
    """Walrus in this toolchain allows at most ONE semaphore wait per
    instruction (and zero on the fused fp32 LDWEIGHTS struct).  Hoist all
    but the last wait of any multi-wait instruction onto same-engine NoOp
    carriers inserted directly before it — semantically identical (engine
    queues are in-order) and each carrier holds a single wait."""
    import bass_rust

    n = 0
    for f in nc.m.functions:
        for bb in f.blocks:
            out_insts = []
            changed = False
            for inst in bb.instructions:
                si = inst.sync_info
                waits = list(si.on_wait) if si is not None and si.on_wait else []
                if len(waits) > 1:
                    changed = True
                    for w in waits[:-1]:
                        nop = bass_rust.InstNoOp(
                            name=f"{inst.name}-sw{n}", engine=inst.engine
                        )
                        n += 1
                        nop.sync_info = mybir.SyncInfo(on_wait=[w], on_update=[])
                        out_insts.append(nop)
                    inst.sync_info = mybir.SyncInfo(
                        on_wait=[waits[-1]],
                        on_update=list(si.on_update) if si.on_update else [],
                    )
                out_insts.append(inst)
            if changed:
                bb.instructions = out_insts
    return nc


def _prepare_in_maps(x, W0, b0, W1, b1):
    x = np.asarray(x, dtype=np.float32)
    xt1 = np.ascontiguousarray(x.T.astype(bfloat16))

    in_maps = []
    for c in range(N_CORES):
        sl = slice(c * GPC, (c + 1) * GPC)
        W0c = np.asarray(W0[sl], dtype=np.float32)  # (256, 64, 64) [g, j, k]
        W1c = np.asarray(W1[sl], dtype=np.float32)
        b0c = np.asarray(b0[sl], dtype=np.float32)  # (256, 64)
        b1c = np.asarray(b1[sl], dtype=np.float32)

        # w0t[k, t, q*64+j] = W0[2t+q, j, k]
        w0 = np.ascontiguousarray(
            W0c.transpose(2, 0, 1).reshape(IN_DIM, NPAIR, 128).astype(bfloat16)
        )

        # w1t[q*64+k, t, q'*64+j] = W1[2t+q, j, k] iff q == q'
        w1k = W1c.transpose(2, 0, 1).reshape(D1, NPAIR, 2, D2)  # [k, t, q, j]
        w1 = np.zeros((2, D1, NPAIR, 2, D2), dtype=bfloat16)
        for q in range(2):
            w1[q, :, :, q, :] = w1k[:, :, q, :].astype(bfloat16)
        w1 = np.ascontiguousarray(w1.reshape(128, NPAIR, 128))

        b0pp = np.ascontiguousarray(b0c.reshape(NPAIR, 128).T)  # (128, NPAIR)
        b1pp = np.ascontiguousarray(b1c.reshape(NPAIR, 128).T)
        in_maps.append(
            {"xt1": xt1, "w0t": w0, "w1t": w1, "b0p": b0pp, "b1p": b1pp}
        )
    return in_maps


def _postprocess(results):
    outs = []
    for c in range(N_CORES):
        o = results[c]["out"]  # (NPAIR, 128, B) f16 = [t, q*64+j, b]
        o = (
            o.astype(np.float32)
            .reshape(NPAIR, 2, D2, B)
            .transpose(3, 0, 1, 2)
            .reshape(B, GPC, D2)
        )
        outs.append(o)
    return np.ascontiguousarray(np.concatenate(outs, axis=1))


def _run(inputs, trace=False):
    nc = _build()
    in_maps = _prepare_in_maps(**inputs)
    res = run_bass_kernel_spmd(
        nc, in_maps, core_ids=list(range(N_CORES)), trace=trace
    )
    return _postprocess(res.results), res


def kernel(x, W0, b0, W1, b1):
    out, _ = _run({"x": x, "W0": W0, "b0": b0, "W1": W1, "b1": b1})
    return out


# revision 9
# speedup vs baseline: 2.6526x; 1.4243x over previous
"""Grouped 2-layer MLP (ConvNN) Trainium2 kernel.

Math (per group g of SIZE=2048):
    h[b,g,:]   = LeakyReLU_0.2(W0[g] @ x[b] + b0[g])     (64 -> 64)
    out[b,g,:] = W1[g] @ h[b,g,:] + b1[g]                (64 -> 64)

Strategy (v5 — row-tiled L0, PE-stream-bound pipeline):
  - This environment's PE streaming clock is pinned at ~1.2GHz (HAM never
    un-throttles; verified: bf16 and f16 N=512 matmuls both take
    512/1.2GHz=427ns, DVE/ACT run at documented clocks).  The moving
    XBUS moves 256B/cycle, so a K=64 bf16 matmul wastes half the bus.
    Layer-0 therefore runs TWO pairs concurrently as row-tiles: even
    pair on array rows 0-63, odd pair on rows 64-127 (x duplicated on
    both partition halves, per-pair W0 stationaries stacked likewise).
    The two streams share the bus perfectly -> L0 time halves.
  - Layer-1 (K=128 block-diagonal) already uses the full bus width.
  - Shard the group axis over 8 cores (256 groups/core = 128 pairs),
    fully independent, no collectives.
  - Per pair t the dataflow is
        mm0 (PE) -> hps (PSUM fp32) -> Prelu (ACT) -> hsb (SBUF f16)
        mm1 (PE) -> ops (PSUM fp32) -> +b1  (DVE) -> osb (SBUF f16) -> DMA
    On TRN2 matmul output must be fp32, so each evacuation pass runs at
    1x (ACT (FD+352)/1.2ns, DVE (FD+120)/0.96ns).  With FD=1024 per-pair
    ops the engine budgets per core are PE 109us / ACT 147us / DVE 153us
    / DMA 108us, all overlapped -> DVE-paced wall ~160us.
  - Layer-0 bias rides the ACT Prelu's per-partition bias operand
    (per-pair ops make that legal), exactly as the proven baseline did.
  - Layer-1 stationary is a host-built 128x128 block-diagonal (2 groups
    per pair); off-diagonal zeros kill cross terms and cost nothing
    since matmul time is N-driven.
  - Weights are host-packed exactly in stationary layout and DMA'd in 8
    chunks of 16 pairs, prefetched 2 chunks ahead and interleaved with
    output DMAs on the FIFO sync ring, so compute starts after ~2us and
    input loads never block the output stream.
  - PE instruction stream is software-pipelined (mm0(t) emitted before
    mm1(t-1)) so the PE never sits behind the ACT dependency; gaps stay
    well under the HAM idle window so the PE runs warm at 2.4GHz.
  - PSUM: 4 rotating [128,1024] fp32 tiles (2 banks each) = all 8 banks:
    hps(t), ops(t-1) per iteration, double-buffered.
  - fp16 output (32MB/core instead of 64) is widened to fp32 on host.
"""

from contextlib import ExitStack

import numpy as np
from ml_dtypes import bfloat16

import concourse.bass as bass
import concourse.mybir as mybir
import concourse.tile as tile
from concourse.bass_utils import run_bass_kernel_spmd

B = 1024
IN_DIM = 64
SIZE = 2048
D1 = 64
D2 = 64
NEG_SLOPE = 0.2
N_CORES = 8
GPC = SIZE // N_CORES  # 256 groups per core
NPAIR = GPC // 2  # 128 group-pairs per core
CH = 16  # pairs per weight DMA chunk
NCH = NPAIR // CH

_NC_CACHE = None
_SIM_RELU = False  # CoreSim has no Prelu; debug builds swap in Relu
_SKIP_SPLIT = False  # sim-only: skip the walrus single-wait workaround


def _build():
    global _NC_CACHE
    if _NC_CACHE is not None:
        return _NC_CACHE

    f32 = mybir.dt.float32
    f16 = mybir.dt.float16
    bf16 = mybir.dt.bfloat16

    nc = bass.Bass()
    xt1 = nc.declare_dram_parameter("xt1", [128, B], bf16, isOutput=False)
    w0t = nc.declare_dram_parameter("w0t", [128, NPAIR // 2, 128], bf16, isOutput=False)
    b0p = nc.declare_dram_parameter("b0p", [128, NPAIR], f32, isOutput=False)
    w1t = nc.declare_dram_parameter("w1t", [128, NPAIR, 128], bf16, isOutput=False)
    b1p = nc.declare_dram_parameter("b1p", [128, NPAIR], f32, isOutput=False)
    out = nc.declare_dram_parameter("out", [NPAIR, 128, B], f16, isOutput=True)

    with ExitStack() as ctx:
        tc = ctx.enter_context(tile.TileContext(nc))
        singles = ctx.enter_context(tc.tile_pool(name="singles", bufs=1))
        hpool = ctx.enter_context(tc.tile_pool(name="hpool", bufs=4))
        opool = ctx.enter_context(tc.tile_pool(name="opool", bufs=4))
        pspool = ctx.enter_context(tc.tile_pool(name="psum", bufs=4, space="PSUM"))

        xt = singles.tile([128, B], bf16)
        nc.sync.dma_start(out=xt, in_=xt1[:])

        w0sb = singles.tile([128, NPAIR // 2, 128], bf16)
        w1sb = singles.tile([128, NPAIR, 128], bf16)

        def load_chunk(k):
            cks = bass.ts(k, CH // 2)
            ck = bass.ts(k, CH)
            nc.sync.dma_start(out=w0sb[:, cks, :], in_=w0t[:, cks, :])
            nc.sync.dma_start(out=w1sb[:, ck, :], in_=w1t[:, ck, :])

        load_chunk(0)
        b0sb = singles.tile([128, NPAIR], f32)
        nc.sync.dma_start(out=b0sb, in_=b0p[:])
        b1sb = singles.tile([128, NPAIR], f32)
        nc.sync.dma_start(out=b1sb, in_=b1p[:])
        load_chunk(1)

        NSUP = NPAIR // 2
        hsbs = [None, None]
        for u in range(NSUP + 1):
            # keep weight chunks ~2 ahead of the consuming pairs
            if (2 * u) % CH == 0 and (2 * u) // CH + 2 < NCH:
                load_chunk((2 * u) // CH + 2)
            if u < NSUP:
                # layer-0: two pairs as concurrent row-tiles (rows 0-63 and
                # 64-127 of the PE array share the moving bus perfectly)
                hpsA = pspool.tile([128, B], f32, tag="ps", name=f"hps{2 * u}")
                hpsB = pspool.tile([128, B], f32, tag="ps", name=f"hps{2 * u + 1}")
                for nb in range(2):
                    s = bass.ts(nb, 512)
                    nc.tensor.matmul(
                        hpsA[:, s], w0sb[0:64, u, :], xt[0:64, s],
                        start=True, stop=True,
                    )
                    nc.tensor.matmul(
                        hpsB[:, s], w0sb[64:128, u, :], xt[64:128, s],
                        start=True, stop=True,
                    )
            if u >= 1:
                opss = []
                for p in (2 * u - 2, 2 * u - 1):
                    ops = pspool.tile([128, B], f32, tag="ps", name=f"ops{p}")
                    for nb in range(2):
                        s = bass.ts(nb, 512)
                        nc.tensor.matmul(
                            ops[:, s], w1sb[:, p, :], hsbs[p % 2][:, s],
                            start=True, stop=True,
                        )
                    opss.append(ops)
            if u < NSUP:
                for i, hps in enumerate((hpsA, hpsB)):
                    t = 2 * u + i
                    hsb_new = hpool.tile([128, B], bf16, tag="h", name=f"hsb{t}")
                    nc.scalar.activation(
                        out=hsb_new,
                        in_=hps,
                        func=mybir.ActivationFunctionType.Relu
                        if _SIM_RELU
                        else mybir.ActivationFunctionType.Prelu,
                        bias=b0sb[:, t : t + 1],
                        scale=1.0,
                        alpha=NEG_SLOPE,
                    )
                    hsbs[i] = hsb_new
            if u >= 1:
                for i, p in enumerate((2 * u - 2, 2 * u - 1)):
                    osb = opool.tile([128, B], f16, tag="o", name=f"osb{p}")
                    nc.vector.tensor_scalar_add(osb, opss[i], b1sb[:, p : p + 1])
                    nc.sync.dma_start(out=out[p], in_=osb)

    if not _SKIP_SPLIT:
        _split_multi_waits(nc)
    _NC_CACHE = nc
    return nc


def _split_multi_waits(nc):
    """Walrus in this toolchain allows at most ONE semaphore wait per
    instruction (and zero on the fused fp32 LDWEIGHTS struct).  Hoist all
    but the last wait of any multi-wait instruction onto same-engine NoOp
    carriers inserted directly before it — semantically identical (engine
    queues are in-order) and each carrier holds a single wait."""
    import bass_rust

    n = 0
    for f in nc.m.functions:
        for bb in f.blocks:
            out_insts = []
            changed = False
            for inst in bb.instructions:
                si = inst.sync_info
                waits = list(si.on_wait) if si is not None and si.on_wait else []
                if len(waits) > 1:
                    changed = True
                    for w in waits[:-1]:
                        nop = bass_rust.InstNoOp(
                            name=f"{inst.name}-sw{n}", engine=inst.engine
                        )
                        n += 1
                        nop.sync_info = mybir.SyncInfo(on_wait=[w], on_update=[])
                        out_insts.append(nop)
                    inst.sync_info = mybir.SyncInfo(
                        on_wait=[waits[-1]],
                        on_update=list(si.on_update) if si.on_update else [],
                    )
                out_insts.append(inst)
            if changed:
                bb.instructions = out_insts
    return nc


def _prepare_in_maps(x, W0, b0, W1, b1):
    x = np.asarray(x, dtype=np.float32)
    xT = x.T.astype(bfloat16)
    xt1 = np.ascontiguousarray(np.concatenate([xT, xT], axis=0))  # (128, B)

    in_maps = []
    for c in range(N_CORES):
        sl = slice(c * GPC, (c + 1) * GPC)
        W0c = np.asarray(W0[sl], dtype=np.float32)  # (256, 64, 64) [g, j, k]
        W1c = np.asarray(W1[sl], dtype=np.float32)
        b0c = np.asarray(b0[sl], dtype=np.float32)  # (256, 64)
        b1c = np.asarray(b1[sl], dtype=np.float32)

        # w0t[64*(t%2)+k, t//2, q*64+j] = W0[2t+q, j, k]  (row-tile stack)
        w0k = W0c.transpose(2, 0, 1).reshape(IN_DIM, NPAIR, 128)
        w0 = np.ascontiguousarray(
            w0k.reshape(IN_DIM, NPAIR // 2, 2, 128)
            .transpose(2, 0, 1, 3)
            .reshape(128, NPAIR // 2, 128)
            .astype(bfloat16)
        )

        # w1t[q*64+k, t, q'*64+j] = W1[2t+q, j, k] iff q == q'
        w1k = W1c.transpose(2, 0, 1).reshape(D1, NPAIR, 2, D2)  # [k, t, q, j]
        w1 = np.zeros((2, D1, NPAIR, 2, D2), dtype=bfloat16)
        for q in range(2):
            w1[q, :, :, q, :] = w1k[:, :, q, :].astype(bfloat16)
        w1 = np.ascontiguousarray(w1.reshape(128, NPAIR, 128))

        b0pp = np.ascontiguousarray(b0c.reshape(NPAIR, 128).T)  # (128, NPAIR)
        b1pp = np.ascontiguousarray(b1c.reshape(NPAIR, 128).T)
        in_maps.append(
            {"xt1": xt1, "w0t": w0, "w1t": w1, "b0p": b0pp, "b1p": b1pp}
        )
    return in_maps


def _postprocess(results):
    outs = []
    for c in range(N_CORES):
        o = results[c]["out"]  # (NPAIR, 128, B) f16 = [t, q*64+j, b]
        o = (
            o.astype(np.float32)
            .reshape(NPAIR, 2, D2, B)
            .transpose(3, 0, 1, 2)
            .reshape(B, GPC, D2)
        )
        outs.append(o)
    return np.ascontiguousarray(np.concatenate(outs, axis=1))


def _run(inputs, trace=False):
    nc = _build()
    in_maps = _prepare_in_maps(**inputs)
    res = run_bass_kernel_spmd(
        nc, in_maps, core_ids=list(range(N_CORES)), trace=trace
    )
    return _postprocess(res.results), res


def kernel(x, W0, b0, W1, b1):
    out, _ = _run({"x": x, "W0": W0, "b0": b0, "W1": W1, "b1": b1})
    return out


# revision 10
# speedup vs baseline: 2.7193x; 1.0251x over previous
"""Grouped 2-layer MLP (ConvNN) Trainium2 kernel.

Math (per group g of SIZE=2048):
    h[b,g,:]   = LeakyReLU_0.2(W0[g] @ x[b] + b0[g])     (64 -> 64)
    out[b,g,:] = W1[g] @ h[b,g,:] + b1[g]                (64 -> 64)

Strategy (v5 — row-tiled L0, PE-stream-bound pipeline):
  - This environment's PE streaming clock is pinned at ~1.2GHz (HAM never
    un-throttles; verified: bf16 and f16 N=512 matmuls both take
    512/1.2GHz=427ns, DVE/ACT run at documented clocks).  The moving
    XBUS moves 256B/cycle, so a K=64 bf16 matmul wastes half the bus.
    Layer-0 therefore runs TWO pairs concurrently as row-tiles: even
    pair on array rows 0-63, odd pair on rows 64-127 (x duplicated on
    both partition halves, per-pair W0 stationaries stacked likewise).
    The two streams share the bus perfectly -> L0 time halves.
  - Layer-1 (K=128 block-diagonal) already uses the full bus width.
  - Shard the group axis over 8 cores (256 groups/core = 128 pairs),
    fully independent, no collectives.
  - Per pair t the dataflow is
        mm0 (PE) -> hps (PSUM fp32) -> Prelu (ACT) -> hsb (SBUF f16)
        mm1 (PE) -> ops (PSUM fp32) -> +b1  (DVE) -> osb (SBUF f16) -> DMA
    On TRN2 matmul output must be fp32, so each evacuation pass runs at
    1x (ACT (FD+352)/1.2ns, DVE (FD+120)/0.96ns).  With FD=1024 per-pair
    ops the engine budgets per core are PE 109us / ACT 147us / DVE 153us
    / DMA 108us, all overlapped -> DVE-paced wall ~160us.
  - Layer-0 bias rides the ACT Prelu's per-partition bias operand
    (per-pair ops make that legal), exactly as the proven baseline did.
  - Layer-1 stationary is a host-built 128x128 block-diagonal (2 groups
    per pair); off-diagonal zeros kill cross terms and cost nothing
    since matmul time is N-driven.
  - Weights are host-packed exactly in stationary layout and DMA'd in 8
    chunks of 16 pairs, prefetched 2 chunks ahead and interleaved with
    output DMAs on the FIFO sync ring, so compute starts after ~2us and
    input loads never block the output stream.
  - PE instruction stream is software-pipelined (mm0(t) emitted before
    mm1(t-1)) so the PE never sits behind the ACT dependency; gaps stay
    well under the HAM idle window so the PE runs warm at 2.4GHz.
  - PSUM: 4 rotating [128,1024] fp32 tiles (2 banks each) = all 8 banks:
    hps(t), ops(t-1) per iteration, double-buffered.
  - fp16 output (32MB/core instead of 64) is widened to fp32 on host.
"""

from contextlib import ExitStack

import numpy as np
from ml_dtypes import bfloat16

import concourse.bass as bass
import concourse.mybir as mybir
import concourse.tile as tile
from concourse.bass_utils import run_bass_kernel_spmd

B = 1024
IN_DIM = 64
SIZE = 2048
D1 = 64
D2 = 64
NEG_SLOPE = 0.2
N_CORES = 8
GPC = SIZE // N_CORES  # 256 groups per core
NPAIR = GPC // 2  # 128 group-pairs per core
CH = 16  # pairs per weight DMA chunk
NCH = NPAIR // CH

_NC_CACHE = None
_SIM_RELU = False  # CoreSim has no Prelu; debug builds swap in Relu
_SKIP_SPLIT = False  # sim-only: skip the walrus single-wait workaround


def _build():
    global _NC_CACHE
    if _NC_CACHE is not None:
        return _NC_CACHE

    f32 = mybir.dt.float32
    f16 = mybir.dt.float16
    bf16 = mybir.dt.bfloat16

    nc = bass.Bass()
    xt1 = nc.declare_dram_parameter("xt1", [128, B], bf16, isOutput=False)
    w0t = nc.declare_dram_parameter("w0t", [128, NPAIR // 2, 128], bf16, isOutput=False)
    b0p = nc.declare_dram_parameter("b0p", [128, NPAIR], f32, isOutput=False)
    w1t = nc.declare_dram_parameter("w1t", [128, NPAIR, 128], bf16, isOutput=False)
    b1p = nc.declare_dram_parameter("b1p", [128, NPAIR], f32, isOutput=False)
    out = nc.declare_dram_parameter("out", [NPAIR, 128, B], f16, isOutput=True)

    with ExitStack() as ctx:
        tc = ctx.enter_context(tile.TileContext(nc))
        singles = ctx.enter_context(tc.tile_pool(name="singles", bufs=1))
        hpool = ctx.enter_context(tc.tile_pool(name="hpool", bufs=4))
        opool = ctx.enter_context(tc.tile_pool(name="opool", bufs=4))
        pspool = ctx.enter_context(tc.tile_pool(name="psum", bufs=4, space="PSUM"))

        xt = singles.tile([128, B], bf16)
        nc.sync.dma_start(out=xt, in_=xt1[:])

        w0sb = singles.tile([128, NPAIR // 2, 128], bf16)
        w1sb = singles.tile([128, NPAIR, 128], bf16)

        bounds = [0, 4, 16, 32, 48, 64, 80, 96, 112, 128]

        def load_chunk(i):
            lo, hi = bounds[i], bounds[i + 1]
            nc.sync.dma_start(
                out=w0sb[:, lo // 2 : hi // 2, :], in_=w0t[:, lo // 2 : hi // 2, :]
            )
            nc.sync.dma_start(out=w1sb[:, lo:hi, :], in_=w1t[:, lo:hi, :])

        load_chunk(0)
        b0sb = singles.tile([128, NPAIR], f32)
        nc.sync.dma_start(out=b0sb, in_=b0p[:])
        b1sb = singles.tile([128, NPAIR], f32)
        nc.sync.dma_start(out=b1sb, in_=b1p[:])
        load_chunk(1)

        NSUP = NPAIR // 2
        next_chunk = 2
        hsbs = [None, None]
        for u in range(NSUP + 1):
            # keep weight loads ~32 pairs ahead of the consuming pairs
            while next_chunk < len(bounds) - 1 and bounds[next_chunk] < 2 * u + 32:
                load_chunk(next_chunk)
                next_chunk += 1
            if u < NSUP:
                # layer-0: two pairs as concurrent row-tiles (rows 0-63 and
                # 64-127 of the PE array share the moving bus perfectly)
                hpsA = pspool.tile([128, B], f32, tag="ps", name=f"hps{2 * u}")
                hpsB = pspool.tile([128, B], f32, tag="ps", name=f"hps{2 * u + 1}")
                for nb in range(2):
                    s = bass.ts(nb, 512)
                    nc.tensor.matmul(
                        hpsA[:, s], w0sb[0:64, u, :], xt[0:64, s],
                        start=True, stop=True,
                    )
                    nc.tensor.matmul(
                        hpsB[:, s], w0sb[64:128, u, :], xt[64:128, s],
                        start=True, stop=True,
                    )
            if u >= 1:
                opss = []
                for p in (2 * u - 2, 2 * u - 1):
                    ops = pspool.tile([128, B], f32, tag="ps", name=f"ops{p}")
                    for nb in range(2):
                        s = bass.ts(nb, 512)
                        nc.tensor.matmul(
                            ops[:, s], w1sb[:, p, :], hsbs[p % 2][:, s],
                            start=True, stop=True,
                        )
                    opss.append(ops)
            if u < NSUP:
                for i, hps in enumerate((hpsA, hpsB)):
                    t = 2 * u + i
                    hsb_new = hpool.tile([128, B], bf16, tag="h", name=f"hsb{t}")
                    nc.scalar.activation(
                        out=hsb_new,
                        in_=hps,
                        func=mybir.ActivationFunctionType.Relu
                        if _SIM_RELU
                        else mybir.ActivationFunctionType.Prelu,
                        bias=b0sb[:, t : t + 1],
                        scale=1.0,
                        alpha=NEG_SLOPE,
                    )
                    hsbs[i] = hsb_new
            if u >= 1:
                for i, p in enumerate((2 * u - 2, 2 * u - 1)):
                    osb = opool.tile([128, B], f16, tag="o", name=f"osb{p}")
                    if p % 16 == 7:
                        # 8/128 pairs evacuate layer-1 on ACT (identity+bias)
                        # to balance ACT (~1.06us/op) vs DVE (~1.19us/op)
                        nc.scalar.add(osb, opss[i], b1sb[:, p : p + 1])
                    else:
                        nc.vector.tensor_scalar_add(osb, opss[i], b1sb[:, p : p + 1])
                    nc.sync.dma_start(out=out[p], in_=osb)

    if not _SKIP_SPLIT:
        _split_multi_waits(nc)
    _NC_CACHE = nc
    return nc


def _split_multi_waits(nc):
    """Walrus in this toolchain allows at most ONE semaphore wait per
    instruction (and zero on the fused fp32 LDWEIGHTS struct).  Hoist all
    but the last wait of any multi-wait instruction onto same-engine NoOp
    carriers inserted directly before it — semantically identical (engine
    queues are in-order) and each carrier holds a single wait."""
    import bass_rust

    n = 0
    for f in nc.m.functions:
        for bb in f.blocks:
            out_insts = []
            changed = False
            for inst in bb.instructions:
                si = inst.sync_info
                waits = list(si.on_wait) if si is not None and si.on_wait else []
                if len(waits) > 1:
                    changed = True
                    for w in waits[:-1]:
                        nop = bass_rust.InstNoOp(
                            name=f"{inst.name}-sw{n}", engine=inst.engine
                        )
                        n += 1
                        nop.sync_info = mybir.SyncInfo(on_wait=[w], on_update=[])
                        out_insts.append(nop)
                    inst.sync_info = mybir.SyncInfo(
                        on_wait=[waits[-1]],
                        on_update=list(si.on_update) if si.on_update else [],
                    )
                out_insts.append(inst)
            if changed:
                bb.instructions = out_insts
    return nc


def _prepare_in_maps(x, W0, b0, W1, b1):
    x = np.asarray(x, dtype=np.float32)
    xT = x.T.astype(bfloat16)
    xt1 = np.ascontiguousarray(np.concatenate([xT, xT], axis=0))  # (128, B)

    in_maps = []
    for c in range(N_CORES):
        sl = slice(c * GPC, (c + 1) * GPC)
        W0c = np.asarray(W0[sl], dtype=np.float32)  # (256, 64, 64) [g, j, k]
        W1c = np.asarray(W1[sl], dtype=np.float32)
        b0c = np.asarray(b0[sl], dtype=np.float32)  # (256, 64)
        b1c = np.asarray(b1[sl], dtype=np.float32)

        # w0t[64*(t%2)+k, t//2, q*64+j] = W0[2t+q, j, k]  (row-tile stack)
        w0k = W0c.transpose(2, 0, 1).reshape(IN_DIM, NPAIR, 128)
        w0 = np.ascontiguousarray(
            w0k.reshape(IN_DIM, NPAIR // 2, 2, 128)
            .transpose(2, 0, 1, 3)
            .reshape(128, NPAIR // 2, 128)
            .astype(bfloat16)
        )

        # w1t[q*64+k, t, q'*64+j] = W1[2t+q, j, k] iff q == q'
        w1k = W1c.transpose(2, 0, 1).reshape(D1, NPAIR, 2, D2)  # [k, t, q, j]
        w1 = np.zeros((2, D1, NPAIR, 2, D2), dtype=bfloat16)
        for q in range(2):
            w1[q, :, :, q, :] = w1k[:, :, q, :].astype(bfloat16)
        w1 = np.ascontiguousarray(w1.reshape(128, NPAIR, 128))

        b0pp = np.ascontiguousarray(b0c.reshape(NPAIR, 128).T)  # (128, NPAIR)
        b1pp = np.ascontiguousarray(b1c.reshape(NPAIR, 128).T)
        in_maps.append(
            {"xt1": xt1, "w0t": w0, "w1t": w1, "b0p": b0pp, "b1p": b1pp}
        )
    return in_maps


def _postprocess(results):
    outs = []
    for c in range(N_CORES):
        o = results[c]["out"]  # (NPAIR, 128, B) f16 = [t, q*64+j, b]
        o = (
            o.astype(np.float32)
            .reshape(NPAIR, 2, D2, B)
            .transpose(3, 0, 1, 2)
            .reshape(B, GPC, D2)
        )
        outs.append(o)
    return np.ascontiguousarray(np.concatenate(outs, axis=1))


def _run(inputs, trace=False):
    nc = _build()
    in_maps = _prepare_in_maps(**inputs)
    res = run_bass_kernel_spmd(
        nc, in_maps, core_ids=list(range(N_CORES)), trace=trace
    )
    return _postprocess(res.results), res


def kernel(x, W0, b0, W1, b1):
    out, _ = _run({"x": x, "W0": W0, "b0": b0, "W1": W1, "b1": b1})
    return out


# revision 11
# speedup vs baseline: 2.7658x; 1.0171x over previous
"""Grouped 2-layer MLP (ConvNN) Trainium2 kernel.

Math (per group g of SIZE=2048):
    h[b,g,:]   = LeakyReLU_0.2(W0[g] @ x[b] + b0[g])     (64 -> 64)
    out[b,g,:] = W1[g] @ h[b,g,:] + b1[g]                (64 -> 64)

Strategy (v5 — row-tiled L0, PE-stream-bound pipeline):
  - This environment's PE streaming clock is pinned at ~1.2GHz (HAM never
    un-throttles; verified: bf16 and f16 N=512 matmuls both take
    512/1.2GHz=427ns, DVE/ACT run at documented clocks).  The moving
    XBUS moves 256B/cycle, so a K=64 bf16 matmul wastes half the bus.
    Layer-0 therefore runs TWO pairs concurrently as row-tiles: even
    pair on array rows 0-63, odd pair on rows 64-127 (x duplicated on
    both partition halves, per-pair W0 stationaries stacked likewise).
    The two streams share the bus perfectly -> L0 time halves.
  - Layer-1 (K=128 block-diagonal) already uses the full bus width.
  - Shard the group axis over 8 cores (256 groups/core = 128 pairs),
    fully independent, no collectives.
  - Per pair t the dataflow is
        mm0 (PE) -> hps (PSUM fp32) -> Prelu (ACT) -> hsb (SBUF f16)
        mm1 (PE) -> ops (PSUM fp32) -> +b1  (DVE) -> osb (SBUF f16) -> DMA
    On TRN2 matmul output must be fp32, so each evacuation pass runs at
    1x (ACT (FD+352)/1.2ns, DVE (FD+120)/0.96ns).  With FD=1024 per-pair
    ops the engine budgets per core are PE 109us / ACT 147us / DVE 153us
    / DMA 108us, all overlapped -> DVE-paced wall ~160us.
  - Layer-0 bias rides the ACT Prelu's per-partition bias operand
    (per-pair ops make that legal), exactly as the proven baseline did.
  - Layer-1 stationary is a host-built 128x128 block-diagonal (2 groups
    per pair); off-diagonal zeros kill cross terms and cost nothing
    since matmul time is N-driven.
  - Weights are host-packed exactly in stationary layout and DMA'd in 8
    chunks of 16 pairs, prefetched 2 chunks ahead and interleaved with
    output DMAs on the FIFO sync ring, so compute starts after ~2us and
    input loads never block the output stream.
  - PE instruction stream is software-pipelined (mm0(t) emitted before
    mm1(t-1)) so the PE never sits behind the ACT dependency; gaps stay
    well under the HAM idle window so the PE runs warm at 2.4GHz.
  - PSUM: 4 rotating [128,1024] fp32 tiles (2 banks each) = all 8 banks:
    hps(t), ops(t-1) per iteration, double-buffered.
  - fp16 output (32MB/core instead of 64) is widened to fp32 on host.
"""

from contextlib import ExitStack

import numpy as np
from ml_dtypes import bfloat16

import concourse.bass as bass
import concourse.mybir as mybir
import concourse.tile as tile
from concourse.bass_utils import run_bass_kernel_spmd

B = 1024
IN_DIM = 64
SIZE = 2048
D1 = 64
D2 = 64
NEG_SLOPE = 0.2
N_CORES = 8
GPC = SIZE // N_CORES  # 256 groups per core
NPAIR = GPC // 2  # 128 group-pairs per core
CH = 16  # pairs per weight DMA chunk
NCH = NPAIR // CH

_NC_CACHE = None
_SIM_RELU = False  # CoreSim has no Prelu; debug builds swap in Relu
_SKIP_SPLIT = False  # sim-only: skip the walrus single-wait workaround


def _build():
    global _NC_CACHE
    if _NC_CACHE is not None:
        return _NC_CACHE

    f32 = mybir.dt.float32
    f16 = mybir.dt.float16
    bf16 = mybir.dt.bfloat16

    nc = bass.Bass()
    xt1 = nc.declare_dram_parameter("xt1", [128, B], bf16, isOutput=False)
    w0t = nc.declare_dram_parameter("w0t", [128, NPAIR // 2, 128], bf16, isOutput=False)
    b0p = nc.declare_dram_parameter("b0p", [128, NPAIR], f32, isOutput=False)
    w1t = nc.declare_dram_parameter("w1t", [128, NPAIR, 128], bf16, isOutput=False)
    b1p = nc.declare_dram_parameter("b1p", [128, NPAIR], f32, isOutput=False)
    out = nc.declare_dram_parameter("out", [NPAIR, 128, B], f16, isOutput=True)

    with ExitStack() as ctx:
        tc = ctx.enter_context(tile.TileContext(nc))
        singles = ctx.enter_context(tc.tile_pool(name="singles", bufs=1))
        hpool = ctx.enter_context(tc.tile_pool(name="hpool", bufs=6))
        opool = ctx.enter_context(tc.tile_pool(name="opool", bufs=6))
        pspool = ctx.enter_context(tc.tile_pool(name="psum", bufs=4, space="PSUM"))

        # Input loads ride idle engines' DMA rings so the sync ring carries
        # ONLY the output stream: startup-critical tensors (xt, first weight
        # chunk, b0) go HWDGE-via-scalar (ACT is idle until the first Prelu);
        # bulk weight chunks + b1 go SWDGE-via-gpsimd (fully idle engine,
        # ~32-pair prefetch lead swallows the higher fixed latency).
        xt = singles.tile([128, B], bf16)
        nc.scalar.dma_start(out=xt, in_=xt1[:])

        w0sb = singles.tile([128, NPAIR // 2, 128], bf16)
        w1sb = singles.tile([128, NPAIR, 128], bf16)

        bounds = [0, 4, 16, 32, 48, 64, 80, 96, 112, 128]

        def load_chunk(i, eng):
            lo, hi = bounds[i], bounds[i + 1]
            eng.dma_start(
                out=w0sb[:, lo // 2 : hi // 2, :], in_=w0t[:, lo // 2 : hi // 2, :]
            )
            eng.dma_start(out=w1sb[:, lo:hi, :], in_=w1t[:, lo:hi, :])

        load_chunk(0, nc.scalar)
        b0sb = singles.tile([128, NPAIR], f32)
        nc.scalar.dma_start(out=b0sb, in_=b0p[:])
        b1sb = singles.tile([128, NPAIR], f32)
        nc.gpsimd.dma_start(out=b1sb, in_=b1p[:])
        load_chunk(1, nc.gpsimd)

        NSUP = NPAIR // 2
        next_chunk = 2
        hsbs = [None, None]
        for u in range(NSUP + 1):
            # keep weight loads ~32 pairs ahead of the consuming pairs
            while next_chunk < len(bounds) - 1 and bounds[next_chunk] < 2 * u + 32:
                load_chunk(next_chunk, nc.gpsimd)
                next_chunk += 1
            if u < NSUP:
                # layer-0: two pairs as concurrent row-tiles (rows 0-63 and
                # 64-127 of the PE array share the moving bus perfectly)
                hpsA = pspool.tile([128, B], f32, tag="ps", name=f"hps{2 * u}")
                hpsB = pspool.tile([128, B], f32, tag="ps", name=f"hps{2 * u + 1}")
                for nb in range(2):
                    s = bass.ts(nb, 512)
                    nc.tensor.matmul(
                        hpsA[:, s], w0sb[0:64, u, :], xt[0:64, s],
                        start=True, stop=True,
                    )
                    nc.tensor.matmul(
                        hpsB[:, s], w0sb[64:128, u, :], xt[64:128, s],
                        start=True, stop=True,
                    )
            if u >= 1:
                opss = []
                for p in (2 * u - 2, 2 * u - 1):
                    ops = pspool.tile([128, B], f32, tag="ps", name=f"ops{p}")
                    for nb in range(2):
                        s = bass.ts(nb, 512)
                        nc.tensor.matmul(
                            ops[:, s], w1sb[:, p, :], hsbs[p % 2][:, s],
                            start=True, stop=True,
                        )
                    opss.append(ops)
            if u < NSUP:
                for i, hps in enumerate((hpsA, hpsB)):
                    t = 2 * u + i
                    hsb_new = hpool.tile([128, B], bf16, tag="h", name=f"hsb{t}")
                    nc.scalar.activation(
                        out=hsb_new,
                        in_=hps,
                        func=mybir.ActivationFunctionType.Relu
                        if _SIM_RELU
                        else mybir.ActivationFunctionType.Prelu,
                        bias=b0sb[:, t : t + 1],
                        scale=1.0,
                        alpha=NEG_SLOPE,
                    )
                    hsbs[i] = hsb_new
            if u >= 1:
                for i, p in enumerate((2 * u - 2, 2 * u - 1)):
                    osb = opool.tile([128, B], f16, tag="o", name=f"osb{p}")
                    if p % 16 == 7:
                        # 8/128 pairs evacuate layer-1 on ACT (identity+bias)
                        # to balance ACT (~1.06us/op) vs DVE (~1.19us/op)
                        nc.scalar.add(osb, opss[i], b1sb[:, p : p + 1])
                    else:
                        nc.vector.tensor_scalar_add(osb, opss[i], b1sb[:, p : p + 1])
                    nc.sync.dma_start(out=out[p], in_=osb)

    if not _SKIP_SPLIT:
        _split_multi_waits(nc)
    _NC_CACHE = nc
    return nc


def _split_multi_waits(nc):
    """Walrus in this toolchain allows at most ONE semaphore wait per
    instruction (and zero on the fused fp32 LDWEIGHTS struct).  Hoist all
    but the last wait of any multi-wait instruction onto same-engine NoOp
    carriers inserted directly before it — semantically identical (engine
    queues are in-order) and each carrier holds a single wait."""
    import bass_rust

    n = 0
    for f in nc.m.functions:
        for bb in f.blocks:
            out_insts = []
            changed = False
            for inst in bb.instructions:
                si = inst.sync_info
                waits = list(si.on_wait) if si is not None and si.on_wait else []
                if len(waits) > 1:
                    changed = True
                    for w in waits[:-1]:
                        nop = bass_rust.InstNoOp(
                            name=f"{inst.name}-sw{n}", engine=inst.engine
                        )
                        n += 1
                        nop.sync_info = mybir.SyncInfo(on_wait=[w], on_update=[])
                        out_insts.append(nop)
                    inst.sync_info = mybir.SyncInfo(
                        on_wait=[waits[-1]],
                        on_update=list(si.on_update) if si.on_update else [],
                    )
                out_insts.append(inst)
            if changed:
                bb.instructions = out_insts
    return nc


def _prepare_in_maps(x, W0, b0, W1, b1):
    x = np.asarray(x, dtype=np.float32)
    xT = x.T.astype(bfloat16)
    xt1 = np.ascontiguousarray(np.concatenate([xT, xT], axis=0))  # (128, B)

    in_maps = []
    for c in range(N_CORES):
        sl = slice(c * GPC, (c + 1) * GPC)
        W0c = np.asarray(W0[sl], dtype=np.float32)  # (256, 64, 64) [g, j, k]
        W1c = np.asarray(W1[sl], dtype=np.float32)
        b0c = np.asarray(b0[sl], dtype=np.float32)  # (256, 64)
        b1c = np.asarray(b1[sl], dtype=np.float32)

        # w0t[64*(t%2)+k, t//2, q*64+j] = W0[2t+q, j, k]  (row-tile stack)
        w0k = W0c.transpose(2, 0, 1).reshape(IN_DIM, NPAIR, 128)
        w0 = np.ascontiguousarray(
            w0k.reshape(IN_DIM, NPAIR // 2, 2, 128)
            .transpose(2, 0, 1, 3)
            .reshape(128, NPAIR // 2, 128)
            .astype(bfloat16)
        )

        # w1t[q*64+k, t, q'*64+j] = W1[2t+q, j, k] iff q == q'
        w1k = W1c.transpose(2, 0, 1).reshape(D1, NPAIR, 2, D2)  # [k, t, q, j]
        w1 = np.zeros((2, D1, NPAIR, 2, D2), dtype=bfloat16)
        for q in range(2):
            w1[q, :, :, q, :] = w1k[:, :, q, :].astype(bfloat16)
        w1 = np.ascontiguousarray(w1.reshape(128, NPAIR, 128))

        b0pp = np.ascontiguousarray(b0c.reshape(NPAIR, 128).T)  # (128, NPAIR)
        b1pp = np.ascontiguousarray(b1c.reshape(NPAIR, 128).T)
        in_maps.append(
            {"xt1": xt1, "w0t": w0, "w1t": w1, "b0p": b0pp, "b1p": b1pp}
        )
    return in_maps


def _postprocess(results):
    outs = []
    for c in range(N_CORES):
        o = results[c]["out"]  # (NPAIR, 128, B) f16 = [t, q*64+j, b]
        o = (
            o.astype(np.float32)
            .reshape(NPAIR, 2, D2, B)
            .transpose(3, 0, 1, 2)
            .reshape(B, GPC, D2)
        )
        outs.append(o)
    return np.ascontiguousarray(np.concatenate(outs, axis=1))


def _run(inputs, trace=False):
    nc = _build()
    in_maps = _prepare_in_maps(**inputs)
    res = run_bass_kernel_spmd(
        nc, in_maps, core_ids=list(range(N_CORES)), trace=trace
    )
    return _postprocess(res.results), res


def kernel(x, W0, b0, W1, b1):
    out, _ = _run({"x": x, "W0": W0, "b0": b0, "W1": W1, "b1": b1})
    return out
